# revision 6
# baseline (speedup 1.0000x reference)
"""Soft-KNN metric regressor forward on 8 Trainium2 NeuronCores.

Reference math (per row n of s, cluster j):
    dist[n,j] = ||s_n||^2 + ||c_j||^2 - 2 s_n.c_j
    w[n,j]    = |cw_j| * exp(-t * dist[n,j])          (t = exp(log_temp))
    mean[n]   = (w @ means) / (sum_j w + 1)
    chol      = diag(exp(log_sigma))

Device formulation (data-parallel over n, 8 cores, no collectives):
    E'[j,n] = exp(2t*(c_j.s_n) - t*||c_j||^2 + ln|cw_j|)   == w[n,j] / r_n
    [num | sig] = [means | 1]^T @ E'                        (GEMM2, j-contraction)
    mean[n] = num[n] * r_n / (sig[n] * r_n + 1),  r_n = exp(-t*||s_n||^2)

GEMM1 (d-contraction) keeps j on PSUM partitions so the per-center term
(-t*||c_j||^2 + ln|cw_j|) rides the ScalarE activation bias and the exp is
fused into the PSUM->SBUF eviction. The per-row factor r_n is a tiny host
precompute ([N] elements) fed as an input; multiplying by it after GEMM2
reproduces the reference's fp32 underflow behavior exactly (r_n = 0 for the
nominal input distribution => output exactly 0, matching jax fp32).
"""

import sys

sys.path.insert(0, "/opt/trn_rl_repo")

import ml_dtypes
import numpy as np

import concourse.bass as bass
import concourse.mybir as mybir
import concourse.tile as tile
from concourse import bacc
from concourse.bass_utils import run_bass_kernel_spmd
from concourse.masks import make_identity

N, K, D, A = 65536, 1024, 256, 64
NCORES = 8
NS = N // NCORES          # rows per core
JT = K // 128             # center partition-tiles
DCH = D // 128            # contraction chunks
SUP = 1024                # n-columns per superblock (2 PSUM banks wide)
AP1 = A + 1               # means columns + ones column

F32 = mybir.dt.float32
BF16 = mybir.dt.bfloat16
BF16_NP = ml_dtypes.bfloat16

_build_cache: dict = {}
last_results = None


def _build_nc(two_t: float, ns: int = NS):
    """Emit the Bass/Tile program for one core's shard (ns rows)."""
    nsup = ns // SUP
    ntiles = ns // 128

    nc = bacc.Bacc("TRN2", target_bir_lowering=False, debug=False,
                   enable_asserts=False)

    sT_d = nc.dram_tensor("sT", [DCH, 128, ns], BF16, kind="ExternalInput")
    # ct: [p, ch*K + j] = centers[j, ch*128+p];  mpp: [p, jt*AP1 + a]
    cT_d = nc.dram_tensor("cT", [128, DCH * K], BF16, kind="ExternalInput")
    mpp_d = nc.dram_tensor("mpp", [128, JT * AP1], BF16, kind="ExternalInput")
    # aux: cols 0:JT = center bias, JT: = r per n-tile
    aux_d = nc.dram_tensor("aux", [128, JT + ntiles], F32, kind="ExternalInput")
    out_d = nc.dram_tensor("out", [ns, A], F32, kind="ExternalOutput")

    Exp = mybir.ActivationFunctionType.Exp

    with tile.TileContext(nc) as tc:
        with (
            tc.tile_pool(name="const", bufs=1) as cpool,
            tc.tile_pool(name="stp", bufs=2 * nsup) as stpool,
            tc.tile_pool(name="epool", bufs=3) as epool,
            tc.tile_pool(name="t1p", bufs=2) as t1pool,
            tc.tile_pool(name="smp", bufs=8) as smpool,
            tc.tile_pool(name="obp", bufs=8) as obpool,
            tc.tile_pool(name="pa", bufs=2, space=bass.MemorySpace.PSUM) as papool,
            tc.tile_pool(name="pb", bufs=1, space=bass.MemorySpace.PSUM) as pbpool,
            tc.tile_pool(name="pc", bufs=2, space=bass.MemorySpace.PSUM) as pcpool,
        ):
            # --- warm the ACT exp table before any data lands ---
            warm = cpool.tile([128, 1], F32, name="warm", tag="warm")
            nc.gpsimd.memset(warm[:], 0.0)
            nc.scalar.activation(warm[:], warm[:], Exp)

            # --- constants + first superblock first, rest of s^T after ---
            ct = cpool.tile([128, DCH * K], BF16, name="ct", tag="ct")
            nc.sync.dma_start(ct[:], cT_d[:, :])

            st = {}

            def load_st(sup, ch):
                t_ = stpool.tile([128, SUP], BF16,
                                 name=f"st_{sup}_{ch}", tag="st")
                nc.sync.dma_start(
                    t_[:], sT_d[ch, :, sup * SUP:(sup + 1) * SUP])
                st[(sup, ch)] = t_

            for ch in range(DCH):
                load_st(0, ch)

            aux = cpool.tile([128, JT + ntiles], F32, name="aux_sb", tag="aux_sb")
            nc.sync.dma_start(aux[:], aux_d[:, :])
            bj = aux[:, 0:JT]
            rt = aux[:, JT:JT + ntiles]
            mpp = cpool.tile([128, JT * AP1], BF16, name="mpp_sb", tag="mpp_sb")
            nc.sync.dma_start(mpp[:], mpp_d[:, :])
            ident = cpool.tile([128, 128], F32, name="ident", tag="ident")
            make_identity(nc, ident[:])

            for sup in range(1, nsup):
                for ch in range(DCH):
                    load_st(sup, ch)

            pb_tiles = {}

            def g1_exp(sup, jt):
                pa = papool.tile([128, SUP], F32, name=f"pa_{sup}_{jt}", tag="pa")
                for ch in range(DCH):
                    lhs = ct[:, ch * K + jt * 128: ch * K + (jt + 1) * 128]
                    for h in range(SUP // 512):
                        nc.tensor.matmul(
                            pa[:, h * 512:(h + 1) * 512],
                            lhs,
                            st[(sup, ch)][:, h * 512:(h + 1) * 512],
                            start=(ch == 0), stop=(ch == DCH - 1))
                e = epool.tile([128, SUP], BF16, name=f"e_{sup}_{jt}", tag="e")
                nc.scalar.activation(e[:], pa[:], Exp,
                                     bias=bj[:, jt:jt + 1], scale=two_t)
                return e

            def g2(sup, jt, e):
                if jt == 0:
                    pb_tiles[sup] = pbpool.tile([128, SUP], F32,
                                                name=f"pb_{sup}", tag="pb")
                pb = pb_tiles[sup]
                for h in range(SUP // 512):
                    nc.tensor.matmul(
                        pb[0:AP1, h * 512:(h + 1) * 512],
                        mpp[:, jt * AP1:(jt + 1) * AP1],
                        e[:, h * 512:(h + 1) * 512],
                        start=(jt == 0), stop=(jt == JT - 1))

            t1_tiles = {}

            def tail_copy(sup):
                pb = pb_tiles[sup]
                for h in range(SUP // 512):
                    t1 = t1pool.tile([128, 512], F32,
                                     name=f"t1_{sup}_{h}", tag="t1")
                    nc.vector.tensor_copy(t1[0:AP1, :],
                                          pb[0:AP1, h * 512:(h + 1) * 512])
                    t1_tiles[(sup, h)] = t1
                del pb_tiles[sup]

            def tail_rest(sup):
                for h in range(SUP // 512):
                    t1 = t1_tiles.pop((sup, h))
                    pc = pcpool.tile([128, 4 * AP1], F32,
                                     name=f"pc_{sup}_{h}", tag="pc")
                    for q in range(4):
                        nc.tensor.transpose(
                            pc[:, q * AP1:(q + 1) * AP1],
                            t1[0:AP1, q * 128:(q + 1) * 128],
                            ident[0:AP1, 0:AP1])
                    for q in range(4):
                        nt = sup * (SUP // 128) + h * 4 + q
                        pq = pc[:, q * AP1:(q + 1) * AP1]
                        r_ap = rt[:, nt:nt + 1]
                        ta = smpool.tile([128, 1], F32,
                                         name=f"ta_{nt}", tag="ta")
                        nc.vector.tensor_mul(ta[:], pq[:, A:AP1], r_ap)
                        nc.vector.tensor_scalar_add(ta[:], ta[:], 1.0)
                        tb = smpool.tile([128, 1], F32,
                                         name=f"tb_{nt}", tag="tb")
                        nc.vector.reciprocal(tb[:], ta[:])
                        nc.vector.tensor_mul(tb[:], tb[:], r_ap)
                        ob = obpool.tile([128, A], F32,
                                         name=f"ob_{nt}", tag="ob")
                        nc.vector.tensor_scalar_mul(ob[:], pq[:, 0:A], tb[:])
                        nc.sync.dma_start(
                            out_d[nt * 128:(nt + 1) * 128, :], ob[:])

            # software-pipelined issue order: G2 trails G1/exp by one step so
            # the PE never sits waiting on ScalarE's exp; tails trail by a
            # superblock.
            seq = [(sup, jt) for sup in range(nsup) for jt in range(JT)]
            pending = []  # (sup, jt, e) awaiting G2 issue
            for i, (sup, jt) in enumerate(seq):
                e = g1_exp(sup, jt)
                if pending:
                    g2(*pending.pop())
                pending.append((sup, jt, e))
                if sup > 0:
                    if jt == 0:
                        tail_copy(sup - 1)
                    elif jt == 1:
                        tail_rest(sup - 1)
            g2(*pending.pop())
            tail_copy(nsup - 1)
            tail_rest(nsup - 1)

    nc.compile()
    return nc


def _prep_inputs(s, centers, c_weights, means, log_temp, ns=NS, ncores=NCORES):
    """Host-side layout prep. Everything here is O(N*D) or smaller."""
    t = float(np.exp(np.float32(log_temp)))
    n = s.shape[0]

    s64 = s.astype(np.float64)
    c64 = centers.astype(np.float64)
    cn = (c64 * c64).sum(-1)                      # ||c_j||^2
    ln_cw = np.log(np.maximum(np.abs(c_weights.astype(np.float64)), 1e-300))
    bias_j = (-t * cn + ln_cw).astype(np.float32)             # [K]
    bj_host = np.ascontiguousarray(bias_j.reshape(JT, 128).T)  # [128, JT]

    mpp_host = np.ascontiguousarray(
        np.concatenate([means.astype(np.float32),
                        np.ones((K, 1), np.float32)], axis=1)
        .astype(BF16_NP).reshape(JT, 128, AP1))

    sn = (s64 * s64).sum(-1)                      # ||s_n||^2
    r = np.exp(-t * sn).astype(np.float32)        # [N]

    sT_full = np.ascontiguousarray(s.astype(BF16_NP).T).reshape(DCH, 128, n)
    cT_host = np.ascontiguousarray(centers.astype(BF16_NP).T).reshape(DCH, 128, K)

    in_maps = []
    for c in range(ncores):
        lo, hi = c * ns, (c + 1) * ns
        in_maps.append({
            "sT": np.ascontiguousarray(sT_full[:, :, lo:hi]),
            "cT": cT_host,
            "mpp": mpp_host,
            "bj": bj_host,
            "rT": np.ascontiguousarray(r[lo:hi].reshape(ns // 128, 128).T),
        })
    return t, in_maps


def kernel(s, centers, c_weights, means, log_sigma, log_temp):
    global last_results
    s = np.asarray(s, np.float32)
    centers = np.asarray(centers, np.float32)
    c_weights = np.asarray(c_weights, np.float32)
    means = np.asarray(means, np.float32)
    log_sigma = np.asarray(log_sigma, np.float32)

    t, in_maps = _prep_inputs(s, centers, c_weights, means, log_temp)

    key = round(2.0 * t, 12)
    if key not in _build_cache:
        _build_cache[key] = _build_nc(2.0 * t)
    nc = _build_cache[key]

    res = run_bass_kernel_spmd(nc, in_maps, core_ids=list(range(NCORES)))
    last_results = res
    mean = np.concatenate([res.results[c]["out"] for c in range(NCORES)], axis=0)

    chol = np.diag(np.exp(log_sigma)).astype(np.float32)
    return mean, chol


# revision 14
# speedup vs baseline: 3.5727x; 3.5727x over previous
"""Soft-KNN metric regressor forward on 8 Trainium2 NeuronCores.

Reference math (per row n of s, cluster j):
    dist[n,j] = ||s_n||^2 + ||c_j||^2 - 2 s_n.c_j
    w[n,j]    = |cw_j| * exp(-t * dist[n,j])          (t = exp(log_temp))
    mean[n]   = (w @ means) / (sum_j w + 1)
    chol      = diag(exp(log_sigma))

Device formulation (data-parallel over n, 8 cores, no collectives):
    E'[j,n] = exp(2t*(c_j.s_n) - t*||c_j||^2 + ln|cw_j|)   == w[n,j] / r_n
    [num | sig] = [means | 1]^T @ E'                        (GEMM2, j-contraction)
    mean[n] = num[n] * r_n / (sig[n] * r_n + 1),  r_n = exp(-t*||s_n||^2)

GEMM1 (d-contraction) keeps j on PSUM partitions so the per-center term
(-t*||c_j||^2 + ln|cw_j|) rides the ScalarE activation bias and the exp is
fused into the PSUM->SBUF eviction. The per-row factor r_n is a tiny host
precompute ([N] elements) fed as an input; multiplying by it after GEMM2
reproduces the reference's fp32 underflow behavior exactly (r_n = 0 for the
nominal input distribution => output exactly 0, matching jax fp32).
"""

import sys

sys.path.insert(0, "/opt/trn_rl_repo")

import ml_dtypes
import numpy as np

import concourse.bass as bass
import concourse.mybir as mybir
import concourse.tile as tile
from concourse import bacc
from concourse.bass_utils import run_bass_kernel_spmd

N, K, D, A = 65536, 1024, 256, 64
NCORES = 8
NS = N // NCORES          # rows per core
JT = K // 128             # center partition-tiles
DCH = D // 128            # contraction chunks
SUP = 1024                # n-columns per superblock (2 PSUM banks wide)
AP1 = A + 1               # means columns + ones column

F32 = mybir.dt.float32
BF16 = mybir.dt.bfloat16
BF16_NP = ml_dtypes.bfloat16

_build_cache: dict = {}
last_results = None


def _build_nc(two_t: float, ns: int = NS, repeats: int = 1):
    """Emit the Bass/Tile program for one core's shard (ns rows).

    repeats>1 re-runs the compute body on the resident inputs (same outputs)
    -- used only for differential device timing in test.py."""
    nsup = ns // SUP
    ntiles = ns // 128

    nc = bacc.Bacc("TRN2", target_bir_lowering=False, debug=False,
                   enable_asserts=False)

    sT_d = nc.dram_tensor("sT", [DCH, 128, ns], BF16, kind="ExternalInput")
    # ct: [p, ch*K + j] = centers[j, ch*128+p];  mpp: [p, jt*AP1 + a]
    cT_d = nc.dram_tensor("cT", [128, DCH * K], BF16, kind="ExternalInput")
    mpp_d = nc.dram_tensor("mpp", [128, JT * AP1], BF16, kind="ExternalInput")
    # aux: cols 0:JT = center bias, JT: = r per n-tile
    aux_d = nc.dram_tensor("aux", [128, JT + ntiles], F32, kind="ExternalInput")
    out_d = nc.dram_tensor("out", [ns, A], F32, kind="ExternalOutput")

    Exp = mybir.ActivationFunctionType.Exp

    with tile.TileContext(nc) as tc:
        with (
            tc.tile_pool(name="const", bufs=1) as cpool,
            tc.tile_pool(name="stp", bufs=2 * nsup) as stpool,
            tc.tile_pool(name="epool", bufs=18) as epool,
            tc.tile_pool(name="smp", bufs=8) as smpool,
            tc.tile_pool(name="obp", bufs=8) as obpool,
            tc.tile_pool(name="pa", bufs=2, space=bass.MemorySpace.PSUM) as papool,
            tc.tile_pool(name="pb", bufs=4, space=bass.MemorySpace.PSUM) as pbpool,
        ):
            # --- warm the ACT exp table before any data lands ---
            warm = cpool.tile([128, 1], F32, name="warm", tag="warm")
            nc.gpsimd.memset(warm[:], 0.0)
            nc.scalar.activation(warm[:], warm[:], Exp)

            # --- constants + first superblock first, rest of s^T after ---
            ct = cpool.tile([128, DCH * K], BF16, name="ct", tag="ct")
            nc.sync.dma_start(ct[:, 0:K], cT_d[:, 0:K])

            st = {}

            def load_st(sup, ch):
                t_ = stpool.tile([128, SUP], BF16,
                                 name=f"st_{sup}_{ch}", tag="st")
                nc.sync.dma_start(
                    t_[:], sT_d[ch, :, sup * SUP:(sup + 1) * SUP])
                st[(sup, ch)] = t_

            for ch in range(DCH):
                load_st(0, ch)
            nc.sync.dma_start(ct[:, K:DCH * K], cT_d[:, K:DCH * K])

            aux = cpool.tile([128, JT + ntiles], F32, name="aux_sb", tag="aux_sb")
            nc.sync.dma_start(aux[:], aux_d[:, :])
            bj = aux[:, 0:JT]
            rt = aux[:, JT:JT + ntiles]
            mpp = cpool.tile([128, JT * AP1], BF16, name="mpp_sb", tag="mpp_sb")
            nc.sync.dma_start(mpp[:], mpp_d[:, :])

            for sup in range(1, nsup):
                for ch in range(DCH):
                    load_st(sup, ch)

            e_tiles = {}

            def g1_exp(sup, jt):
                pa = papool.tile([128, SUP], F32, name=f"pa_{sup}_{jt}", tag="pa")
                for ch in range(DCH):
                    lhs = ct[:, ch * K + jt * 128: ch * K + (jt + 1) * 128]
                    for h in range(SUP // 512):
                        nc.tensor.matmul(
                            pa[:, h * 512:(h + 1) * 512],
                            lhs,
                            st[(sup, ch)][:, h * 512:(h + 1) * 512],
                            start=(ch == 0), stop=(ch == DCH - 1))
                e = epool.tile([128, SUP], BF16, name=f"e_{sup}_{jt}", tag="e")
                nc.scalar.activation(e[:], pa[:], Exp,
                                     bias=bj[:, jt:jt + 1], scale=two_t)
                e_tiles[(sup, jt)] = e

            def g2_tail(sup, ntl):
                # out2[n, a] for one 128-row tile: E' slices are the PE
                # stationary (65-col streams), result lands n-major so the
                # per-row normalization needs no transpose.
                nt = sup * (SUP // 128) + ntl
                pbv = pbpool.tile([128, AP1], F32, name=f"pb_{nt}", tag="pb")
                for jt in range(JT):
                    nc.tensor.matmul(
                        pbv[:],
                        e_tiles[(sup, jt)][:, ntl * 128:(ntl + 1) * 128],
                        mpp[:, jt * AP1:(jt + 1) * AP1],
                        start=(jt == 0), stop=(jt == JT - 1))
                r_ap = rt[:, nt:nt + 1]
                ta = smpool.tile([128, 1], F32, name=f"ta_{nt}", tag="ta")
                nc.vector.tensor_mul(ta[:], pbv[:, A:AP1], r_ap)
                nc.vector.tensor_scalar_add(ta[:], ta[:], 1.0)
                tb = smpool.tile([128, 1], F32, name=f"tb_{nt}", tag="tb")
                nc.vector.reciprocal(tb[:], ta[:])
                nc.vector.tensor_mul(tb[:], tb[:], r_ap)
                ob = obpool.tile([128, A], F32, name=f"ob_{nt}", tag="ob")
                nc.vector.tensor_scalar_mul(ob[:], pbv[:, 0:A], tb[:])
                nc.sync.dma_start(out_d[nt * 128:(nt + 1) * 128, :], ob[:])

            # G1/exp stream through the supers; each G2 n-tile of the
            # previous super slots between two G1 steps so PE never waits
            # on ScalarE.
            for _rep in range(repeats):
                for sup in range(nsup):
                    for jt in range(JT):
                        g1_exp(sup, jt)
                        if sup > 0:
                            g2_tail(sup - 1, jt)
                for ntl in range(SUP // 128):
                    g2_tail(nsup - 1, ntl)

    nc.compile()
    return nc


def _prep_inputs(s, centers, c_weights, means, log_temp, ns=NS, ncores=NCORES):
    """Host-side layout prep. Everything here is O(N*D) or smaller."""
    t = float(np.exp(np.float32(log_temp)))
    n = s.shape[0]

    s64 = s.astype(np.float64)
    c64 = centers.astype(np.float64)
    cn = (c64 * c64).sum(-1)                      # ||c_j||^2
    ln_cw = np.log(np.maximum(np.abs(c_weights.astype(np.float64)), 1e-300))
    bias_j = (-t * cn + ln_cw).astype(np.float32)             # [K]
    bj_host = np.ascontiguousarray(bias_j.reshape(JT, 128).T)  # [128, JT]

    mpp_host = np.ascontiguousarray(
        np.concatenate([means.astype(np.float32),
                        np.ones((K, 1), np.float32)], axis=1)
        .astype(BF16_NP).reshape(JT, 128, AP1).transpose(1, 0, 2)
        .reshape(128, JT * AP1))

    sn = (s64 * s64).sum(-1)                      # ||s_n||^2
    r = np.exp(-t * sn).astype(np.float32)        # [N]

    sT_full = np.ascontiguousarray(s.astype(BF16_NP).T).reshape(DCH, 128, n)
    cf = np.ascontiguousarray(centers.astype(BF16_NP).T)       # [D, K]
    cT_host = np.ascontiguousarray(
        cf.reshape(DCH, 128, K).transpose(1, 0, 2).reshape(128, DCH * K))

    in_maps = []
    for c in range(ncores):
        lo, hi = c * ns, (c + 1) * ns
        rt_core = np.ascontiguousarray(r[lo:hi].reshape(ns // 128, 128).T)
        in_maps.append({
            "sT": np.ascontiguousarray(sT_full[:, :, lo:hi]),
            "cT": cT_host,
            "mpp": mpp_host,
            "aux": np.ascontiguousarray(
                np.concatenate([bj_host, rt_core], axis=1)),
        })
    return t, in_maps


def kernel(s, centers, c_weights, means, log_sigma, log_temp):
    global last_results
    s = np.asarray(s, np.float32)
    centers = np.asarray(centers, np.float32)
    c_weights = np.asarray(c_weights, np.float32)
    means = np.asarray(means, np.float32)
    log_sigma = np.asarray(log_sigma, np.float32)

    t, in_maps = _prep_inputs(s, centers, c_weights, means, log_temp)

    key = round(2.0 * t, 12)
    if key not in _build_cache:
        _build_cache[key] = _build_nc(2.0 * t)
    nc = _build_cache[key]

    res = run_bass_kernel_spmd(nc, in_maps, core_ids=list(range(NCORES)))
    last_results = res
    mean = np.concatenate([res.results[c]["out"] for c in range(NCORES)], axis=0)

    chol = np.diag(np.exp(log_sigma)).astype(np.float32)
    return mean, chol


# revision 15
# speedup vs baseline: 41.0015x; 11.4764x over previous
"""Soft-KNN metric regressor forward on 8 Trainium2 NeuronCores.

Reference math (per row n of s, cluster j):
    dist[n,j] = ||s_n||^2 + ||c_j||^2 - 2 s_n.c_j
    w[n,j]    = |cw_j| * exp(-t * dist[n,j])          (t = exp(log_temp))
    mean[n]   = (w @ means) / (sum_j w + 1)
    chol      = diag(exp(log_sigma))

Device formulation (data-parallel over n, 8 cores, no collectives):
    E'[j,n] = exp(2t*(c_j.s_n) - t*||c_j||^2 + ln|cw_j|)   == w[n,j] / r_n
    [num | sig] = [means | 1]^T @ E'                        (GEMM2, j-contraction)
    mean[n] = num[n] * r_n / (sig[n] * r_n + 1),  r_n = exp(-t*||s_n||^2)

GEMM1 (d-contraction) keeps j on PSUM partitions so the per-center term
(-t*||c_j||^2 + ln|cw_j|) rides the ScalarE activation bias and the exp is
fused into the PSUM->SBUF eviction. The per-row factor r_n is a tiny host
precompute ([N] elements) fed as an input; multiplying by it after GEMM2
reproduces the reference's fp32 underflow behavior exactly (r_n = 0 for the
nominal input distribution => output exactly 0, matching jax fp32).
"""

import sys

sys.path.insert(0, "/opt/trn_rl_repo")

import ml_dtypes
import numpy as np

import concourse.bass as bass
import concourse.mybir as mybir
import concourse.tile as tile
from concourse import bacc
from concourse.bass_utils import run_bass_kernel_spmd

N, K, D, A = 65536, 1024, 256, 64
NCORES = 8
NS = N // NCORES          # rows per core
JT = K // 128             # center partition-tiles
DCH = D // 128            # contraction chunks
SUP = 1024                # n-columns per superblock (2 PSUM banks wide)
AP1 = A + 1               # means columns + ones column

F32 = mybir.dt.float32
BF16 = mybir.dt.bfloat16
BF16_NP = ml_dtypes.bfloat16

_build_cache: dict = {}
last_results = None


def _build_nc(two_t: float, ns: int = NS, repeats: int = 1):
    """Emit the Bass/Tile program for one core's shard (ns rows).

    repeats>1 re-runs the compute body on the resident inputs (same outputs)
    -- used only for differential device timing in test.py."""
    nsup = ns // SUP
    ntiles = ns // 128

    nc = bacc.Bacc("TRN2", target_bir_lowering=False, debug=False,
                   enable_asserts=False)

    sT_d = nc.dram_tensor("sT", [DCH, 128, ns], BF16, kind="ExternalInput")
    # ct: [p, ch*K + j] = centers[j, ch*128+p];  mpp: [p, jt*AP1 + a]
    cT_d = nc.dram_tensor("cT", [128, DCH * K], BF16, kind="ExternalInput")
    mpp_d = nc.dram_tensor("mpp", [128, JT * AP1], BF16, kind="ExternalInput")
    # aux: cols 0:JT = center bias, JT: = r per n-tile
    aux_d = nc.dram_tensor("aux", [128, JT + ntiles], F32, kind="ExternalInput")
    out_d = nc.dram_tensor("out", [ns, A], F32, kind="ExternalOutput")

    Exp = mybir.ActivationFunctionType.Exp

    with tile.TileContext(nc) as tc:
        with (
            tc.tile_pool(name="const", bufs=1) as cpool,
            tc.tile_pool(name="stp", bufs=2 * nsup) as stpool,
            tc.tile_pool(name="epool", bufs=18) as epool,
            tc.tile_pool(name="smp", bufs=8) as smpool,
            tc.tile_pool(name="obp", bufs=8) as obpool,
            tc.tile_pool(name="pa", bufs=2, space=bass.MemorySpace.PSUM) as papool,
            tc.tile_pool(name="pb", bufs=4, space=bass.MemorySpace.PSUM) as pbpool,
        ):
            # --- warm the ACT exp table before any data lands ---
            warm = cpool.tile([128, 1], F32, name="warm", tag="warm")
            nc.gpsimd.memset(warm[:], 0.0)
            nc.scalar.activation(warm[:], warm[:], Exp)

            # --- constants + first superblock first, rest of s^T after ---
            ct = cpool.tile([128, DCH * K], BF16, name="ct", tag="ct")
            nc.sync.dma_start(ct[:, 0:K], cT_d[:, 0:K])

            st = {}

            def load_st(sup, ch):
                t_ = stpool.tile([128, SUP], BF16,
                                 name=f"st_{sup}_{ch}", tag="st")
                nc.sync.dma_start(
                    t_[:], sT_d[ch, :, sup * SUP:(sup + 1) * SUP])
                st[(sup, ch)] = t_

            for ch in range(DCH):
                load_st(0, ch)
            nc.sync.dma_start(ct[:, K:DCH * K], cT_d[:, K:DCH * K])

            aux = cpool.tile([128, JT + ntiles], F32, name="aux_sb", tag="aux_sb")
            nc.sync.dma_start(aux[:], aux_d[:, :])
            bj = aux[:, 0:JT]
            rt = aux[:, JT:JT + ntiles]
            mpp = cpool.tile([128, JT * AP1], BF16, name="mpp_sb", tag="mpp_sb")
            nc.sync.dma_start(mpp[:], mpp_d[:, :])

            for sup in range(1, nsup):
                for ch in range(DCH):
                    load_st(sup, ch)

            e_tiles = {}

            def g1_exp(sup, jt):
                pa = papool.tile([128, SUP], F32, name=f"pa_{sup}_{jt}", tag="pa")
                for ch in range(DCH):
                    lhs = ct[:, ch * K + jt * 128: ch * K + (jt + 1) * 128]
                    for h in range(SUP // 512):
                        nc.tensor.matmul(
                            pa[:, h * 512:(h + 1) * 512],
                            lhs,
                            st[(sup, ch)][:, h * 512:(h + 1) * 512],
                            start=(ch == 0), stop=(ch == DCH - 1))
                e = epool.tile([128, SUP], BF16, name=f"e_{sup}_{jt}", tag="e")
                nc.scalar.activation(e[:], pa[:], Exp,
                                     bias=bj[:, jt:jt + 1], scale=two_t)
                e_tiles[(sup, jt)] = e

            def g2_tail(sup, ntl):
                # out2[n, a] for one 128-row tile: E' slices are the PE
                # stationary (65-col streams), result lands n-major so the
                # per-row normalization needs no transpose.
                nt = sup * (SUP // 128) + ntl
                pbv = pbpool.tile([128, AP1], F32, name=f"pb_{nt}", tag="pb")
                for jt in range(JT):
                    nc.tensor.matmul(
                        pbv[:],
                        e_tiles[(sup, jt)][:, ntl * 128:(ntl + 1) * 128],
                        mpp[:, jt * AP1:(jt + 1) * AP1],
                        start=(jt == 0), stop=(jt == JT - 1))
                r_ap = rt[:, nt:nt + 1]
                ta = smpool.tile([128, 1], F32, name=f"ta_{nt}", tag="ta")
                # ta = sig*r + 1
                nc.vector.tensor_scalar(ta[:], pbv[:, A:AP1], r_ap, 1.0,
                                        mybir.AluOpType.mult,
                                        mybir.AluOpType.add)
                tb = smpool.tile([128, 1], F32, name=f"tb_{nt}", tag="tb")
                nc.vector.reciprocal(tb[:], ta[:])
                ob = obpool.tile([128, A], F32, name=f"ob_{nt}", tag="ob")
                # ob = num * (1/(sig*r+1)) * r
                nc.vector.tensor_scalar(ob[:], pbv[:, 0:A], tb[:], r_ap,
                                        mybir.AluOpType.mult,
                                        mybir.AluOpType.mult)
                nc.sync.dma_start(out_d[nt * 128:(nt + 1) * 128, :], ob[:])

            # G1/exp stream through the supers; each G2 n-tile of the
            # previous super slots between two G1 steps so PE never waits
            # on ScalarE.
            for _rep in range(repeats):
                for sup in range(nsup):
                    for jt in range(JT):
                        g1_exp(sup, jt)
                        if sup > 0:
                            g2_tail(sup - 1, jt)
                for ntl in range(SUP // 128):
                    g2_tail(nsup - 1, ntl)

    nc.compile()
    return nc


def _prep_inputs(s, centers, c_weights, means, log_temp, ns=NS, ncores=NCORES):
    """Host-side layout prep. Everything here is O(N*D) or smaller."""
    t = float(np.exp(np.float32(log_temp)))
    n = s.shape[0]

    s64 = s.astype(np.float64)
    c64 = centers.astype(np.float64)
    cn = (c64 * c64).sum(-1)                      # ||c_j||^2
    ln_cw = np.log(np.maximum(np.abs(c_weights.astype(np.float64)), 1e-300))
    bias_j = (-t * cn + ln_cw).astype(np.float32)             # [K]
    bj_host = np.ascontiguousarray(bias_j.reshape(JT, 128).T)  # [128, JT]

    mpp_host = np.ascontiguousarray(
        np.concatenate([means.astype(np.float32),
                        np.ones((K, 1), np.float32)], axis=1)
        .astype(BF16_NP).reshape(JT, 128, AP1).transpose(1, 0, 2)
        .reshape(128, JT * AP1))

    sn = (s64 * s64).sum(-1)                      # ||s_n||^2
    r = np.exp(-t * sn).astype(np.float32)        # [N]

    sT_full = np.ascontiguousarray(s.astype(BF16_NP).T).reshape(DCH, 128, n)
    cf = np.ascontiguousarray(centers.astype(BF16_NP).T)       # [D, K]
    cT_host = np.ascontiguousarray(
        cf.reshape(DCH, 128, K).transpose(1, 0, 2).reshape(128, DCH * K))

    in_maps = []
    for c in range(ncores):
        lo, hi = c * ns, (c + 1) * ns
        rt_core = np.ascontiguousarray(r[lo:hi].reshape(ns // 128, 128).T)
        in_maps.append({
            "sT": np.ascontiguousarray(sT_full[:, :, lo:hi]),
            "cT": cT_host,
            "mpp": mpp_host,
            "aux": np.ascontiguousarray(
                np.concatenate([bj_host, rt_core], axis=1)),
        })
    return t, in_maps


def kernel(s, centers, c_weights, means, log_sigma, log_temp):
    global last_results
    s = np.asarray(s, np.float32)
    centers = np.asarray(centers, np.float32)
    c_weights = np.asarray(c_weights, np.float32)
    means = np.asarray(means, np.float32)
    log_sigma = np.asarray(log_sigma, np.float32)

    t, in_maps = _prep_inputs(s, centers, c_weights, means, log_temp)

    key = round(2.0 * t, 12)
    if key not in _build_cache:
        _build_cache[key] = _build_nc(2.0 * t)
    nc = _build_cache[key]

    res = run_bass_kernel_spmd(nc, in_maps, core_ids=list(range(NCORES)))
    last_results = res
    mean = np.concatenate([res.results[c]["out"] for c in range(NCORES)], axis=0)

    chol = np.diag(np.exp(log_sigma)).astype(np.float32)
    return mean, chol
